# revision 68
# baseline (speedup 1.0000x reference)
"""GCN (2-layer) + edge-dot decode on 8 TRN2 NeuronCores — v7.

Math (per GCN layer, dinv = rsqrt(indeg+1)):
    out[v] = dinv[v] * ( sum_{e: dst=v} hs[src_e] + hs[v] ) + b,  hs = dinv (.) (x @ W)

Structure (vs the v2 baseline, 1455us -> ~1295us):
  * GEMM1 is SHARDED: each core computes hs only for its own NPC nodes
    (own xT panel, 3.2MB read) and the full hs table is assembled by a
    Shared-scratchpad HBM AllGather (233GB/s idle vs 45GB/s plain, ends
    ~105us vs ~173us for the old replicated GEMM + table write).
  * All collective outputs use addr_space="Shared" (the fast HBM-HBM
    AllGather path).
  * Aggregation blocks run in REVERSE degree order (48..0, low-degree
    first) so AllGather pieces covering ~50% of rows trigger at ~50% of
    the phase; CC staging DMAs are emitted STAGE_AHEAD blocks before the
    trigger so the gpsimd doorbell write never stalls the gather stream.
  * ag2 -> tab2 in 2 pieces (split at block 19); ag3 -> ztabA (blocks
    14-48) + ztabB (0-13) as SEPARATE tensors so decode's gathers get
    exact-granularity deps (DRAM dep tracking is per-tensor; one ztab
    tensor made decode wait for ALL ag3 pieces). Decode runs in 4
    endpoint-zone groups AA/AB/BA/BB; the first AA chunk is emitted
    before the ag3B trigger so decode gathers overlap that collective.
  * Gathers (HID=128 bf16 rows for tab gathers, OUT=64 f32 rows for
    decode, 256B each) rotate over the 4 SWDGE queues; throughput is
    bounded ~72GB/s by gpsimd descriptor generation (~29ns/row/core),
    which is the dominant term (~870us busy).

dma_gather indices are SIGNED int16 for the big tables (frame centered at
row 32768); ztabB (14336 rows) uses plain positive indices. Every gather
appends one all-positive pad round so a trailing run of real negative
indices is never dropped by the ucode.
"""

import sys
import numpy as np
from contextlib import ExitStack

sys.path.insert(0, "/opt/trn_rl_repo")

import concourse.bass as bass
import concourse.mybir as mybir
from concourse.bass_utils import run_bass_kernel_spmd
from concourse.tile import TileContext, add_dep_helper
from concourse.masks import make_identity
from concourse.library_config import mlp
from concourse.library_overlay import lower_extended_insts

N, E, L = 50000, 800000, 200000
IN, HID, OUT = 256, 128, 64
C = 8                      # cores
NP = 50176                 # padded node count = 392 blocks of 128
NPC = NP // C              # 6272 nodes per core
BPC = NPC // 128           # 49 blocks per core
FBASE = 32768              # gather frame base row (signed int16 centered)
PADIDX = NP - 1 - FBASE    # pad index -> row 50175 (zero pad-node row)
CH_MAX = 24                # max rounds per gather chunk (excl. appended pad round)
DEC_CHUNK = 21             # decode chunk rounds

# Blocks are processed in REVERSE order (48..0, low-degree first) so that
# CC pieces covering most rows trigger early while the compute tail (few
# high-degree blocks) is small.
# tab2 pieces: P1 = blocks 24-48 (done first), P2 = 8-23, P3 = 0-7.
# tab2 region order is [P3][P2][P1] so the zero pad rows (block 48 of
# core 7) land at the very top (>= FBASE, positive in the signed frame).
P1_O = 3840                # o >= P1_O -> ag2 piece 1 (blocks 30-48, ~29% of rounds)
P2_O = 1536                # P2_O <= o < P1_O -> piece 2 (blocks 12-29, ~66%)
R3_BASE = 0                # tab2 region offsets: [P3][P2][P1]
R2_BASE = C * P2_O         # 4096
R1_BASE = R2_BASE + C * (P1_O - P2_O)   # 28672

# ztab zones: A = blocks 14-48 (done first in reverse order), B = 0-13.
ZB_O = 1792                # o < ZB_O -> zone B
ZA_H = NPC - ZB_O          # 4480 rows/core in ztabA
NZA = C * ZA_H             # 35840 rows in ztabA
NZB = C * ZB_O             # 14336 rows in ztabB
PADZA = NZA - 1 - FBASE    # positive pad index in ztabA frame
PADZB = 0                  # positive pad index in ztabB frame
STAGE_AHEAD = 2            # emit CC staging this many blocks before the trigger

BF16 = mybir.dt.bfloat16

CUSTOM_ISA_OPCODES = {"DMAGatherAnt", "DMAScatterAddAnt", "ISA"}


def _fix_sync_waits(nc):
    """This container's walrus accepts at most one sync-wait per instruction
    and none on custom ISA ucode ops; hoist extras onto preceding drains."""
    f = nc.m.functions[0]
    for b in f.blocks:
        insts = b.instructions
        i = 0
        while i < len(insts):
            ins = insts[i]
            si = ins.sync_info
            nw = len(si.on_wait) if (si is not None and si.on_wait is not None) else 0
            keep = 0 if str(ins.opcode) in CUSTOM_ISA_OPCODES else 1
            if nw > keep:
                waits = list(si.on_wait)
                hoist, keepw = waits[: nw - keep], waits[nw - keep:]
                for j, w in enumerate(hoist):
                    d = mybir.InstEventSemaphore(name=f"{ins.name}-wsplit{j}")
                    d.engine = ins.engine
                    d.sync_info = mybir.SyncInfo(on_wait=[w], on_update=[])
                    insts.insert(i + j, d)
                si.on_wait = keepw
                i += len(hoist)
            i += 1


def _sortedpos(p):
    """final position -> position in the degree-sorted sequence."""
    core = p // NPC
    k = (p % NPC) // 128
    lane = p % 128
    return 128 * (8 * k + core) + lane


def _rowmap2(p):
    """final position -> tab2 row, regions [P3][P2][P1]."""
    c = p // NPC
    o = p % NPC
    return np.where(o < P2_O, R3_BASE + c * P2_O + o,
           np.where(o < P1_O, R2_BASE + c * (P1_O - P2_O) + (o - P2_O),
                    R1_BASE + c * (NPC - P1_O) + (o - P1_O)))


def _zzone(p):
    """final position -> 0 if in ztabA (o >= ZB_O), 1 if in ztabB."""
    return (p % NPC) < ZB_O


def _rowmapz(p):
    """final position -> row within its ztab zone tensor."""
    c = p // NPC
    o = p % NPC
    return np.where(o >= ZB_O, c * ZA_H + (o - ZB_O), c * ZB_O + o)


def _wrap_idx(flat):
    """[n] int16 -> [128, n//16] wrapped in 16 partitions, replicated x8."""
    n = flat.shape[0]
    arr = np.empty((16, n // 16), dtype=np.int16)
    arr[:, :] = flat.reshape(n // 16, 16).T
    return np.tile(arr, (8, 1))


def _chunked(total, chmax):
    out = []
    r = 0
    while r < total:
        ch = min(chmax, total - r)
        out.append((r, ch))
        r += ch
    return out


def _prepare(edge_index, edge_label_index):
    src = np.asarray(edge_index[0], dtype=np.int64)
    dst = np.asarray(edge_index[1], dtype=np.int64)
    la = np.asarray(edge_label_index[0], dtype=np.int64)
    lb = np.asarray(edge_label_index[1], dtype=np.int64)

    deg = np.bincount(dst, minlength=N).astype(np.int64)

    # permutation: degree-sorted, core-striped; 176 zero pad nodes at the tail
    sorted_real = np.argsort(-deg, kind="stable")
    seq = np.full(NP, -1, dtype=np.int64)
    seq[:N] = sorted_real
    final_perm = seq[_sortedpos(np.arange(NP))]   # final position -> orig (-1 pad)
    real_mask = final_perm >= 0
    invpos = np.full(N, -1, dtype=np.int64)
    invpos[final_perm[real_mask]] = np.nonzero(real_mask)[0]
    assert final_perm[NP - 1] == -1

    ps = invpos[src]
    pd = invpos[dst]

    # per-node in-edge ranks (dst-major)
    order = np.argsort(pd, kind="stable")
    pd_s = pd[order]
    ps_s = ps[order]
    newgrp = np.empty(E, dtype=bool)
    newgrp[0] = True
    newgrp[1:] = pd_s[1:] != pd_s[:-1]
    gidx = np.nonzero(newgrp)[0]
    rank = np.arange(E) - gidx[np.cumsum(newgrp) - 1]

    lane = pd_s % 128
    core = pd_s // NPC
    slot = (pd_s % NPC) // 128

    nblocks = NP // 128
    KB = np.zeros(nblocks, dtype=np.int64)
    np.maximum.at(KB, pd_s // 128, rank + 1)
    Khat = np.zeros(BPC, dtype=np.int64)
    for k in range(BPC):
        Khat[k] = int(KB[[c * BPC + k for c in range(C)]].max())
    off = np.zeros(BPC + 1, dtype=np.int64)
    off[1:] = np.cumsum(Khat)

    # [core, round-slot, lane] source tables: phase B (tabHS identity rows)
    # and phase C (tab2 zone rows); same rank schedule for both.
    idxB = np.full((C, off[-1], 128), PADIDX, dtype=np.int16)
    idxC = np.full((C, off[-1], 128), PADIDX, dtype=np.int16)
    gslot = off[slot] + rank
    idxB[core, gslot, lane] = (ps_s - FBASE).astype(np.int16)
    idxC[core, gslot, lane] = (_rowmap2(ps_s) - FBASE).astype(np.int16)

    # chunk schedule per block (shared by B and C)
    chunks = []   # (k, r0, ch)
    for k in range(BPC):
        for (r0, ch) in _chunked(int(Khat[k]), CH_MAX):
            chunks.append((k, r0, ch))

    # degp per core [128, BPC]
    degp = np.full(NP, 1e30, dtype=np.float32)
    degp[real_mask] = (deg[final_perm[real_mask]] + 1).astype(np.float32)
    degp_core = degp.reshape(C, BPC, 128).transpose(0, 2, 1).copy()

    # ---- decode: 4 endpoint-zone groups AA, AB, BA, BB ----
    pa = invpos[la]
    pb = invpos[lb]
    LPC = L // C
    za = _zzone(pa)
    zb = _zzone(pb)
    grp_of = za.astype(np.int64) * 2 + zb.astype(np.int64)  # 0=AA 1=AB 2=BA 3=BB
    grp_lists = []   # per core: list of 4 arrays of local label idx
    for c in range(C):
        s = slice(c * LPC, (c + 1) * LPC)
        g = grp_of[s]
        grp_lists.append([np.nonzero(g == i)[0] for i in range(4)])
    rounds_g = []
    chunks_g = []
    for i in range(4):
        r = (max(len(gl[i]) for gl in grp_lists) + 127) // 128
        rounds_g.append(r)
        chunks_g.append(_chunked(r, DEC_CHUNK))

    # per-core slot map: label i (within core slice) -> flat output slot
    dec_slots = []
    for c in range(C):
        sl = np.empty(LPC, dtype=np.int64)
        base = 0
        for i in range(4):
            lst = grp_lists[c][i]
            sl[lst] = base + np.arange(len(lst))
            base += rounds_g[i] * 128
        dec_slots.append(sl)

    def zidx(pos):
        """final positions -> (int16 idx in the proper ztab frame)."""
        zz = _zzone(pos)
        r = _rowmapz(pos)
        return np.where(zz, r, r - FBASE).astype(np.int16)

    padrowA = np.full(128, PADZA, dtype=np.int16)
    padrowB = np.full(128, PADZB, dtype=np.int16)

    def dec_group_tables(c, i, lst):
        s0 = c * LPC
        rounds = rounds_g[i]
        az, bz = i // 2, i % 2
        fa = np.full(rounds * 128, PADZA if az == 0 else PADZB, dtype=np.int16)
        fb = np.full(rounds * 128, PADZA if bz == 0 else PADZB, dtype=np.int16)
        fa[:len(lst)] = zidx(pa[s0 + lst])
        fb[:len(lst)] = zidx(pb[s0 + lst])
        parts = []
        for (r0, ch) in chunks_g[i]:
            parts.append(_wrap_idx(np.concatenate(
                [fa[r0 * 128:(r0 + ch) * 128], padrowA if az == 0 else padrowB])))
            parts.append(_wrap_idx(np.concatenate(
                [fb[r0 * 128:(r0 + ch) * 128], padrowA if bz == 0 else padrowB])))
        return parts

    padrow = np.full(128, PADIDX, dtype=np.int16)
    idx16 = []
    for c in range(C):
        parts = []
        for (k, r0, ch) in chunks:
            flat = idxB[c, off[k] + r0: off[k] + r0 + ch].reshape(-1)
            parts.append(_wrap_idx(np.concatenate([flat, padrow])))
        for (k, r0, ch) in chunks:
            flat = idxC[c, off[k] + r0: off[k] + r0 + ch].reshape(-1)
            parts.append(_wrap_idx(np.concatenate([flat, padrow])))
        for i in range(4):
            parts += dec_group_tables(c, i, grp_lists[c][i])
        idx16.append(np.ascontiguousarray(np.concatenate(parts, axis=1)))

    return dict(
        final_perm=final_perm, invpos=invpos, real_mask=real_mask,
        Khat=Khat, off=off, chunks=chunks,
        chunks_g=chunks_g, rounds_g=rounds_g,
        dec_slots=dec_slots, degp=degp, degp_core=degp_core, idx16=idx16,
    )


def _build(prep):
    chunks = prep["chunks"]
    chunks_g = prep["chunks_g"]
    TOTW = prep["idx16"][0].shape[1]
    ndec_cols = sum(prep["rounds_g"])               # slot s at (s%128, s//128)

    nc = bass.Bass(num_devices=C, dynamic_dma_scratch_size=32768, num_swdge_queues=4)
    f32 = mybir.dt.float32
    xT_d = nc.dram_tensor("xT", [IN, NPC], BF16, kind="ExternalInput")
    W1_d = nc.dram_tensor("W1", [IN, HID], BF16, kind="ExternalInput")
    b1_d = nc.dram_tensor("b1", [1, HID], f32, kind="ExternalInput")
    W2_d = nc.dram_tensor("W2", [HID, OUT], BF16, kind="ExternalInput")
    b2_d = nc.dram_tensor("b2", [1, OUT], f32, kind="ExternalInput")
    degp_d = nc.dram_tensor("degp", [128, BPC], f32, kind="ExternalInput")
    idx_d = nc.dram_tensor("idx16", [128, TOTW], mybir.dt.int16, kind="ExternalInput")
    out_d = nc.dram_tensor("out", [128, ndec_cols], f32, kind="ExternalOutput")

    aghs_in = nc.dram_tensor("aghs_in", [NPC, HID], BF16)
    tabHS = nc.dram_tensor("tabHS", [NP, HID], BF16, addr_space="Shared")
    ag2_in = nc.dram_tensor("ag2_in", [NPC, HID], BF16)
    tab2 = nc.dram_tensor("tab2", [NP, HID], BF16, addr_space="Shared")
    ag3_in = nc.dram_tensor("ag3_in", [NPC, OUT], f32)
    ztabA = nc.dram_tensor("ztabA", [NZA, OUT], f32, addr_space="Shared")
    ztabB = nc.dram_tensor("ztabB", [NZB, OUT], f32, addr_space="Shared")

    grp8 = [list(range(C))]

    with TileContext(nc) as tc, ExitStack() as ctx:
        const = ctx.enter_context(tc.tile_pool(name="const", bufs=1))
        own = ctx.enter_context(tc.tile_pool(name="own", bufs=1))
        xp_pool = ctx.enter_context(tc.tile_pool(name="xpan", bufs=2))
        gp = ctx.enter_context(tc.tile_pool(name="gath", bufs=5))
        dgp = ctx.enter_context(tc.tile_pool(name="dgath", bufs=3))
        ppa = ctx.enter_context(tc.tile_pool(name="psA", bufs=2, space="PSUM"))
        pp = ctx.enter_context(tc.tile_pool(name="psAgg", bufs=3, space="PSUM"))
        pz = ctx.enter_context(tc.tile_pool(name="psZ", bufs=1, space="PSUM"))
        sp_ = ctx.enter_context(tc.tile_pool(name="stage", bufs=4))
        prp = ctx.enter_context(tc.tile_pool(name="prodp", bufs=2))

        ll = nc.gpsimd.load_library(mlp)

        ident = const.tile([128, 128], BF16)
        make_identity(nc, ident[:])
        identf = const.tile([128, 128], mybir.dt.float32)
        make_identity(nc, identf[:])

        idx_sb = const.tile([128, TOTW], mybir.dt.int16)
        idma = nc.sync.dma_start(out=idx_sb[:], in_=idx_d[:, :])
        add_dep_helper(idma.ins, ll.ins, reason="idx after lib load")

        kvals = sorted({(ch + 1) * 128 for (_, _, ch) in chunks}
                       | {(ch + 1) * 128 for cg in chunks_g for (_, ch) in cg})
        kreg = {}
        for v in kvals:
            r = ctx.enter_context(nc.gpsimd.register(f"nidx{v}"))
            nc.gpsimd.reg_mov(r, v)
            kreg[v] = r

        W1_sb = []
        for i in range(2):
            w1t = const.tile([128, HID], BF16, tag=f"w1_{i}", name=f"w1_{i}")
            nc.sync.dma_start(out=w1t[:], in_=W1_d[i * 128:(i + 1) * 128, :])
            W1_sb.append(w1t)
        W2_sb = const.tile([128, OUT], BF16)
        nc.sync.dma_start(out=W2_sb[:], in_=W2_d[:, :])

        ones_row = const.tile([1, 128], f32)
        nc.vector.memset(ones_row[:], 1.0)
        b1_row = const.tile([1, HID], f32)
        nc.sync.dma_start(out=b1_row[:], in_=b1_d[:, :])
        b2_row = const.tile([1, OUT], f32)
        nc.sync.dma_start(out=b2_row[:], in_=b2_d[:, :])
        bias1 = const.tile([128, HID], f32)
        bps = pz.tile([128, HID], f32, tag="qt")
        nc.tensor.matmul(out=bps[:], lhsT=ones_row[:], rhs=b1_row[:], start=True, stop=True)
        nc.scalar.activation(out=bias1[:], in_=bps[:], func=mybir.ActivationFunctionType.Copy)
        bias2 = const.tile([128, OUT], f32)
        bps2 = pz.tile([128, OUT], f32, tag="qt")
        nc.tensor.matmul(out=bps2[:], lhsT=ones_row[:], rhs=b2_row[:], start=True, stop=True)
        nc.scalar.activation(out=bias2[:], in_=bps2[:], func=mybir.ActivationFunctionType.Copy)

        degp_sb = const.tile([128, BPC], f32)
        nc.sync.dma_start(out=degp_sb[:], in_=degp_d[:, :])
        rec = const.tile([128, BPC], f32)
        nc.vector.reciprocal(out=rec[:], in_=degp_sb[:])
        dinv = const.tile([128, BPC], f32)
        nc.scalar.activation(out=dinv[:], in_=rec[:], func=mybir.ActivationFunctionType.Sqrt)

        hs_own = own.tile([128, NPC], BF16)
        g_own = own.tile([128, NPC], BF16)
        z_own = own.tile([128, NPC // 2], f32)   # 49 blocks x 64 cols

        # chunk -> idx column offsets (B group, C group, decode groups)
        co = 0
        blk_chunksB = [[] for _ in range(BPC)]
        for (k, r0, ch) in chunks:
            blk_chunksB[k].append((co, ch))
            co += (ch + 1) * 8
        blk_chunksC = [[] for _ in range(BPC)]
        for (k, r0, ch) in chunks:
            blk_chunksC[k].append((co, ch))
            co += (ch + 1) * 8
        dec_coffs = []   # per group: list of col offsets (a at coff, b adjacent)
        for i in range(4):
            offs = []
            for (r0, ch) in chunks_g[i]:
                offs.append(co)
                co += 2 * (ch + 1) * 8
            dec_coffs.append(offs)
        assert co == TOTW

        # ---------------- Phase A: own-panel GEMM1 -> AllGather hs ----------------
        GW = 3200
        with nc.named_scope("gemm1"):
            for (o0, nb) in ((0, 25), (GW, BPC - 25)):
                x0 = xp_pool.tile([128, GW], BF16, tag="x0", name="x0")
                nc.sync.dma_start(out=x0[:, :nb * 128],
                                  in_=xT_d[0:128, o0: o0 + nb * 128])
                x1 = xp_pool.tile([128, GW], BF16, tag="x1", name="x1")
                nc.sync.dma_start(out=x1[:, :nb * 128],
                                  in_=xT_d[128:256, o0: o0 + nb * 128])
                for q0 in range(0, nb, 4):
                    G = min(4, nb - q0)
                    ps = ppa.tile([128, 512], f32, tag="a", name="psa")
                    for g in range(G):
                        cs = slice((q0 + g) * 128, (q0 + g + 1) * 128)
                        nc.tensor.matmul(out=ps[:, g * 128:(g + 1) * 128],
                                         lhsT=x0[:, cs], rhs=W1_sb[0][:],
                                         start=True, stop=False)
                        nc.tensor.matmul(out=ps[:, g * 128:(g + 1) * 128],
                                         lhsT=x1[:, cs], rhs=W1_sb[1][:],
                                         start=False, stop=True)
                    dst = hs_own[:, o0 + q0 * 128: o0 + (q0 + G) * 128]
                    if (q0 // 4) % 2 == 0:
                        nc.scalar.activation(out=dst, in_=ps[:, :G * 128],
                                             func=mybir.ActivationFunctionType.Copy)
                    else:
                        nc.vector.tensor_copy(out=dst, in_=ps[:, :G * 128])
            whs = nc.sync.dma_start(
                out=aghs_in[:, :].rearrange("(b l) f -> l b f", l=128),
                in_=hs_own[:].rearrange("p (b f) -> p b f", f=HID))
            with nc.named_scope("aghs"):
                cc_hs = nc.gpsimd.collective_compute(
                    "AllGather", mybir.AluOpType.bypass,
                    replica_groups=grp8,
                    ins=[aghs_in[:, :].opt()], outs=[tabHS[:, :].opt()])
                add_dep_helper(cc_hs.ins, whs.ins, reason="aghs")

        qctr = [0]

        def aggregate(k, tab, blk_chunks, own_tile, dep_ins):
            ps = pp.tile([128, HID], f32, tag="main", name="psagg")
            first = True
            for ci, (coff, ch) in enumerate(blk_chunks[k]):
                gt = gp.tile([128, CH_MAX + 1, HID], BF16, tag="gt", name="gt")
                qctr[0] = (qctr[0] + 1) % 4
                gi = nc.gpsimd.dma_gather(
                    gt[:, :ch + 1, :], tab[FBASE:, :],
                    idx_sb[:, coff:coff + (ch + 1) * 8],
                    (ch + 1) * 128, kreg[(ch + 1) * 128], HID, single_packet=False,
                    queue_num=qctr[0])
                for d in dep_ins:
                    add_dep_helper(gi.ins, d.ins, reason="gather after table ready")
                for r in range(ch):
                    nc.tensor.matmul(out=ps[:], lhsT=ident[:], rhs=gt[:, r, :],
                                     start=first, stop=False)
                    first = False
            nc.tensor.matmul(out=ps[:], lhsT=ident[:],
                             rhs=own_tile[:, k * 128:(k + 1) * 128],
                             start=first, stop=True)
            return ps

        # ---------------- Phase B: layer-1 aggregation (tabHS) -> g ----------------
        # Blocks run 48..0 (low-degree first) so ag2 pieces launch early.
        ccs2 = []

        def ag2_stage(o0, o1):
            return nc.sync.dma_start(
                out=ag2_in[o0:o1, :].rearrange("(b l) f -> l b f", l=128),
                in_=g_own[:, o0:o1].rearrange("p (b f) -> p b f", f=HID))

        def ag2_trigger(o0, o1, obase, name, w):
            with nc.named_scope(name):
                cc = nc.gpsimd.collective_compute(
                    "AllGather", mybir.AluOpType.bypass,
                    replica_groups=grp8,
                    ins=[ag2_in[o0:o1, :].opt()],
                    outs=[tab2[obase:obase + C * (o1 - o0), :].opt()])
                add_dep_helper(cc.ins, w.ins, reason=name)
                ccs2.append(cc)

        with nc.named_scope("agg1"):
            w2p1 = None
            for i, k in enumerate(reversed(range(BPC))):
                dep = [cc_hs] if i == 0 else []
                ps = aggregate(k, tabHS, blk_chunksB, hs_own, dep)
                t1 = sp_.tile([128, HID], f32, tag="t1", name="t1")
                nc.scalar.activation(out=t1[:], in_=ps[:],
                                     func=mybir.ActivationFunctionType.Copy,
                                     scale=dinv[:, k:k + 1])
                t2 = sp_.tile([128, HID], f32, tag="t2", name="t2")
                nc.vector.tensor_add(out=t2[:], in0=t1[:], in1=bias1[:])
                nc.scalar.activation(out=g_own[:, k * 128:(k + 1) * 128], in_=t2[:],
                                     func=mybir.ActivationFunctionType.Relu,
                                     scale=dinv[:, k:k + 1])
                if k == P1_O // 128 + STAGE_AHEAD:    # stage piece 1 early
                    w2p1 = ag2_stage(P1_O + STAGE_AHEAD * 128, NPC)
                if k == P1_O // 128:                  # blocks 48..28 done
                    w2p1b = ag2_stage(P1_O, P1_O + STAGE_AHEAD * 128)
                    add_dep_helper(w2p1b.ins, w2p1.ins, reason="order stage")
                    ag2_trigger(P1_O, NPC, R1_BASE, "ag2p1", w2p1b)
                if k == P2_O // 128 + STAGE_AHEAD:    # stage piece 2 early
                    w2p2 = ag2_stage(P2_O + STAGE_AHEAD * 128, P1_O)
                if k == P2_O // 128:                  # blocks 27..4 done
                    w2p2b = ag2_stage(P2_O, P2_O + STAGE_AHEAD * 128)
                    add_dep_helper(w2p2b.ins, w2p2.ins, reason="order stage")
                    ag2_trigger(P2_O, P1_O, R2_BASE, "ag2p2", w2p2b)
            w2p3 = ag2_stage(0, P2_O)
            ag2_trigger(0, P2_O, R3_BASE, "ag2p3", w2p3)

        # ---------------- Phase C: layer-2 aggregation + GEMM2 -> z ----------------
        ccs3 = []
        with nc.named_scope("agg2"):
            for i, k in enumerate(reversed(range(BPC))):
                dep = ccs2 if i == 0 else []
                ps = aggregate(k, tab2, blk_chunksC, g_own, dep)
                q = sp_.tile([128, HID], f32, tag="q", name="q")
                nc.scalar.activation(out=q[:], in_=ps[:],
                                     func=mybir.ActivationFunctionType.Copy,
                                     scale=dinv[:, k:k + 1])
                qt_ps = pz.tile([128, HID], f32, tag="qt", name="qtps")
                nc.tensor.transpose(out=qt_ps[:], in_=q[:], identity=identf[:])
                qt = sp_.tile([128, HID], BF16, tag="qt_sb", name="qtsb")
                nc.vector.tensor_copy(out=qt[:], in_=qt_ps[:])
                zps = pz.tile([128, OUT], f32, tag="z", name="zps")
                nc.tensor.matmul(out=zps[:], lhsT=qt[:], rhs=W2_sb[:], start=True, stop=True)
                nc.vector.tensor_add(out=z_own[:, k * OUT:(k + 1) * OUT],
                                     in0=zps[:], in1=bias2[:])
                if k == ZB_O // 128 + STAGE_AHEAD:    # stage most of ztabA early
                    w3a = nc.sync.dma_start(
                        out=ag3_in[ZB_O + STAGE_AHEAD * 128:NPC, :].rearrange(
                            "(b l) f -> l b f", l=128),
                        in_=z_own[:, (ZB_O // 128 + STAGE_AHEAD) * OUT:].rearrange(
                            "p (b f) -> p b f", f=OUT))
                if k == ZB_O // 128:          # blocks 48..14 done -> ztabA piece
                    w3ab = nc.sync.dma_start(
                        out=ag3_in[ZB_O:ZB_O + STAGE_AHEAD * 128, :].rearrange(
                            "(b l) f -> l b f", l=128),
                        in_=z_own[:, (ZB_O // 128) * OUT:
                                  (ZB_O // 128 + STAGE_AHEAD) * OUT].rearrange(
                            "p (b f) -> p b f", f=OUT))
                    add_dep_helper(w3ab.ins, w3a.ins, reason="order stage")
                    with nc.named_scope("ag3a"):
                        cc = nc.gpsimd.collective_compute(
                            "AllGather", mybir.AluOpType.bypass,
                            replica_groups=grp8,
                            ins=[ag3_in[ZB_O:NPC, :].opt()],
                            outs=[ztabA[:, :].opt()])
                        add_dep_helper(cc.ins, w3ab.ins, reason="ag3a")
                        ccs3.append(cc)
            w3b = nc.sync.dma_start(
                out=ag3_in[0:ZB_O, :].rearrange("(b l) f -> l b f", l=128),
                in_=z_own[:, :(ZB_O // 128) * OUT].rearrange("p (b f) -> p b f", f=OUT))

        # ---------------- Phase D: decode (4 endpoint-zone groups) ----------------
        # Emit the first AA chunk's gathers BEFORE the ag3b trigger so its
        # staging wait overlaps decode gather time.
        with nc.named_scope("decode"):
            out_sb = own.tile([128, ndec_cols], f32)

            def dec_chunk(i, ci, col, deps_a=(), deps_b=()):
                (r0, ch) = chunks_g[i][ci]
                coff = dec_coffs[i][ci]
                az, bz = i // 2, i % 2
                taba = ztabA[FBASE:, :] if az == 0 else ztabB[0:, :]
                tabb = ztabA[FBASE:, :] if bz == 0 else ztabB[0:, :]
                qa = (2 * ci) % 4
                qb = (2 * ci + 1) % 4
                za_t = dgp.tile([128, DEC_CHUNK + 1, OUT], f32, tag="za", name="za")
                ga = nc.gpsimd.dma_gather(
                    za_t[:, :ch + 1, :], taba,
                    idx_sb[:, coff:coff + (ch + 1) * 8],
                    (ch + 1) * 128, kreg[(ch + 1) * 128], OUT, single_packet=False,
                    queue_num=qa)
                for dcc in deps_a:
                    add_dep_helper(ga.ins, dcc.ins, reason="dec a after ztab")
                zb_t = dgp.tile([128, DEC_CHUNK + 1, OUT], f32, tag="zb", name="zb")
                gb = nc.gpsimd.dma_gather(
                    zb_t[:, :ch + 1, :], tabb,
                    idx_sb[:, coff + (ch + 1) * 8:coff + 2 * (ch + 1) * 8],
                    (ch + 1) * 128, kreg[(ch + 1) * 128], OUT, single_packet=False,
                    queue_num=qb)
                for dcc in deps_b:
                    add_dep_helper(gb.ins, dcc.ins, reason="dec b after ztab")
                prod = prp.tile([128, ch * OUT], f32, tag="prod", name="prod")
                nc.vector.tensor_mul(out=prod[:].rearrange("p (c o) -> p c o", o=OUT),
                                     in0=za_t[:, :ch, :], in1=zb_t[:, :ch, :])
                nc.vector.reduce_sum(out=out_sb[:, col:col + ch],
                                     in_=prod[:].rearrange("p (c o) -> p c o", o=OUT),
                                     axis=mybir.AxisListType.X)
                return col + ch

            with nc.named_scope("ag3b"):
                cc = nc.gpsimd.collective_compute(
                    "AllGather", mybir.AluOpType.bypass,
                    replica_groups=grp8,
                    ins=[ag3_in[0:ZB_O, :].opt()], outs=[ztabB[:, :].opt()])
                add_dep_helper(cc.ins, w3b.ins, reason="ag3b")
                ccs3.append(cc)

            col = 0
            # first AA chunk: explicit deps on the ztabA piece (not ztabB)
            col = dec_chunk(0, 0, col, deps_a=(ccs3[0],), deps_b=(ccs3[0],))
            for ci in range(1, len(chunks_g[0])):
                col = dec_chunk(0, ci, col)
            first_b = True
            for i in (1, 2, 3):
                for ci in range(len(chunks_g[i])):
                    db = (ccs3[1],) if first_b else ()
                    col = dec_chunk(i, ci, col, deps_b=db)
                    first_b = False
            nc.sync.dma_start(out=out_d[:, :], in_=out_sb[:])

    lower_extended_insts(nc)
    _fix_sync_waits(nc)
    return nc


def kernel(x, W1, b1, W2, b2, edge_index, edge_label_index):
    x = np.asarray(x, dtype=np.float32)
    W1 = np.asarray(W1, dtype=np.float32)
    b1 = np.asarray(b1, dtype=np.float32)
    W2 = np.asarray(W2, dtype=np.float32)
    b2 = np.asarray(b2, dtype=np.float32)
    prep = _prepare(np.asarray(edge_index), np.asarray(edge_label_index))
    nc = _build(prep)

    # host-prescaled x: xp[pos] = dinv[pos] * x[perm[pos]]  (pads stay zero)
    xp = np.zeros((NP, IN), dtype=np.float32)
    rm = prep["real_mask"]
    xp[rm] = x[prep["final_perm"][rm]]
    dinv_full = 1.0 / np.sqrt(prep["degp"])
    xp *= dinv_full[:, None]

    import ml_dtypes

    def to_bf16(a):
        return np.asarray(a, dtype=np.float32).astype(ml_dtypes.bfloat16)

    in_maps = []
    for c in range(C):
        xc = xp[c * NPC:(c + 1) * NPC]
        in_maps.append({
            "xT": to_bf16(np.ascontiguousarray(xc.T)),
            "W1": to_bf16(W1), "b1": b1.reshape(1, HID),
            "W2": to_bf16(W2), "b2": b2.reshape(1, OUT),
            "degp": prep["degp_core"][c],
            "idx16": prep["idx16"][c],
        })
    res = run_bass_kernel_spmd(nc, in_maps, core_ids=list(range(C)))

    LPC = L // C
    out = np.empty(L, dtype=np.float32)
    for c in range(C):
        o = res.results[c]["out"]          # [128, ncols]; slot s at (s%128, s//128)
        sl = prep["dec_slots"][c]
        out[c * LPC:(c + 1) * LPC] = o[sl % 128, sl // 128]
    return out


# revision 69
# speedup vs baseline: 1.0295x; 1.0295x over previous
"""GCN (2-layer) + edge-dot decode on 8 TRN2 NeuronCores — v7.

Math (per GCN layer, dinv = rsqrt(indeg+1)):
    out[v] = dinv[v] * ( sum_{e: dst=v} hs[src_e] + hs[v] ) + b,  hs = dinv (.) (x @ W)

Structure (vs the v2 baseline, 1455us -> ~1295us):
  * GEMM1 is SHARDED: each core computes hs only for its own NPC nodes
    (own xT panel, 3.2MB read) and the full hs table is assembled by a
    Shared-scratchpad HBM AllGather (233GB/s idle vs 45GB/s plain, ends
    ~105us vs ~173us for the old replicated GEMM + table write).
  * All collective outputs use addr_space="Shared" (the fast HBM-HBM
    AllGather path).
  * Aggregation blocks run in REVERSE degree order (48..0, low-degree
    first) so AllGather pieces covering ~50% of rows trigger at ~50% of
    the phase; CC staging DMAs are emitted STAGE_AHEAD blocks before the
    trigger so the gpsimd doorbell write never stalls the gather stream.
  * ag2 -> tab2 in 2 pieces (split at block 19); ag3 -> ztabA (blocks
    14-48) + ztabB (0-13) as SEPARATE tensors so decode's gathers get
    exact-granularity deps (DRAM dep tracking is per-tensor; one ztab
    tensor made decode wait for ALL ag3 pieces). Decode runs in 4
    endpoint-zone groups AA/AB/BA/BB; the first AA chunk is emitted
    before the ag3B trigger so decode gathers overlap that collective.
  * Gathers (HID=128 bf16 rows for tab gathers, OUT=64 f32 rows for
    decode, 256B each) rotate over the 4 SWDGE queues; throughput is
    bounded ~72GB/s by gpsimd descriptor generation (~29ns/row/core),
    which is the dominant term (~870us busy).

dma_gather indices are SIGNED int16 for the big tables (frame centered at
row 32768); ztabB (14336 rows) uses plain positive indices. Every gather
appends one all-positive pad round so a trailing run of real negative
indices is never dropped by the ucode.
"""

import sys
import numpy as np
from contextlib import ExitStack

sys.path.insert(0, "/opt/trn_rl_repo")

import concourse.bass as bass
import concourse.mybir as mybir
from concourse.bass_utils import run_bass_kernel_spmd
from concourse.tile import TileContext, add_dep_helper
from concourse.masks import make_identity
from concourse.library_config import mlp
from concourse.library_overlay import lower_extended_insts

N, E, L = 50000, 800000, 200000
IN, HID, OUT = 256, 128, 64
C = 8                      # cores
NP = 50176                 # padded node count = 392 blocks of 128
NPC = NP // C              # 6272 nodes per core
BPC = NPC // 128           # 49 blocks per core
FBASE = 32768              # gather frame base row (signed int16 centered)
PADIDX = NP - 1 - FBASE    # pad index -> row 50175 (zero pad-node row)
CH_MAX = 24                # max rounds per gather chunk (excl. appended pad round)
DEC_CHUNK = 20             # decode chunk rounds

# Blocks are processed in REVERSE order (48..0, low-degree first) so that
# CC pieces covering most rows trigger early while the compute tail (few
# high-degree blocks) is small.
# tab2 pieces: P1 = blocks 24-48 (done first), P2 = 8-23, P3 = 0-7.
# tab2 region order is [P3][P2][P1] so the zero pad rows (block 48 of
# core 7) land at the very top (>= FBASE, positive in the signed frame).
P1_O = 3840                # o >= P1_O -> ag2 piece 1 (blocks 30-48, ~29% of rounds)
P2_O = 1280                # P2_O <= o < P1_O -> piece 2 (blocks 10-29, ~72%)
R3_BASE = 0                # tab2 region offsets: [P3][P2][P1]
R2_BASE = C * P2_O         # 4096
R1_BASE = R2_BASE + C * (P1_O - P2_O)   # 28672

# ztab zones: A = blocks 14-48 (done first in reverse order), B = 0-13.
ZB_O = 1792                # o < ZB_O -> zone B
ZA_H = NPC - ZB_O          # 4480 rows/core in ztabA
NZA = C * ZA_H             # 35840 rows in ztabA
NZB = C * ZB_O             # 14336 rows in ztabB
PADZA = NZA - 1 - FBASE    # positive pad index in ztabA frame
PADZB = 0                  # positive pad index in ztabB frame
STAGE_AHEAD = 2            # emit CC staging this many blocks before the trigger

BF16 = mybir.dt.bfloat16

CUSTOM_ISA_OPCODES = {"DMAGatherAnt", "DMAScatterAddAnt", "ISA"}


def _fix_sync_waits(nc):
    """This container's walrus accepts at most one sync-wait per instruction
    and none on custom ISA ucode ops; hoist extras onto preceding drains."""
    f = nc.m.functions[0]
    for b in f.blocks:
        insts = b.instructions
        i = 0
        while i < len(insts):
            ins = insts[i]
            si = ins.sync_info
            nw = len(si.on_wait) if (si is not None and si.on_wait is not None) else 0
            keep = 0 if str(ins.opcode) in CUSTOM_ISA_OPCODES else 1
            if nw > keep:
                waits = list(si.on_wait)
                hoist, keepw = waits[: nw - keep], waits[nw - keep:]
                for j, w in enumerate(hoist):
                    d = mybir.InstEventSemaphore(name=f"{ins.name}-wsplit{j}")
                    d.engine = ins.engine
                    d.sync_info = mybir.SyncInfo(on_wait=[w], on_update=[])
                    insts.insert(i + j, d)
                si.on_wait = keepw
                i += len(hoist)
            i += 1


def _sortedpos(p):
    """final position -> position in the degree-sorted sequence."""
    core = p // NPC
    k = (p % NPC) // 128
    lane = p % 128
    return 128 * (8 * k + core) + lane


def _rowmap2(p):
    """final position -> tab2 row, regions [P3][P2][P1]."""
    c = p // NPC
    o = p % NPC
    return np.where(o < P2_O, R3_BASE + c * P2_O + o,
           np.where(o < P1_O, R2_BASE + c * (P1_O - P2_O) + (o - P2_O),
                    R1_BASE + c * (NPC - P1_O) + (o - P1_O)))


def _zzone(p):
    """final position -> 0 if in ztabA (o >= ZB_O), 1 if in ztabB."""
    return (p % NPC) < ZB_O


def _rowmapz(p):
    """final position -> row within its ztab zone tensor."""
    c = p // NPC
    o = p % NPC
    return np.where(o >= ZB_O, c * ZA_H + (o - ZB_O), c * ZB_O + o)


def _wrap_idx(flat):
    """[n] int16 -> [128, n//16] wrapped in 16 partitions, replicated x8."""
    n = flat.shape[0]
    arr = np.empty((16, n // 16), dtype=np.int16)
    arr[:, :] = flat.reshape(n // 16, 16).T
    return np.tile(arr, (8, 1))


def _chunked(total, chmax):
    out = []
    r = 0
    while r < total:
        ch = min(chmax, total - r)
        out.append((r, ch))
        r += ch
    return out


def _prepare(edge_index, edge_label_index):
    src = np.asarray(edge_index[0], dtype=np.int64)
    dst = np.asarray(edge_index[1], dtype=np.int64)
    la = np.asarray(edge_label_index[0], dtype=np.int64)
    lb = np.asarray(edge_label_index[1], dtype=np.int64)

    deg = np.bincount(dst, minlength=N).astype(np.int64)

    # permutation: degree-sorted, core-striped; 176 zero pad nodes at the tail
    sorted_real = np.argsort(-deg, kind="stable")
    seq = np.full(NP, -1, dtype=np.int64)
    seq[:N] = sorted_real
    final_perm = seq[_sortedpos(np.arange(NP))]   # final position -> orig (-1 pad)
    real_mask = final_perm >= 0
    invpos = np.full(N, -1, dtype=np.int64)
    invpos[final_perm[real_mask]] = np.nonzero(real_mask)[0]
    assert final_perm[NP - 1] == -1

    ps = invpos[src]
    pd = invpos[dst]

    # per-node in-edge ranks (dst-major)
    order = np.argsort(pd, kind="stable")
    pd_s = pd[order]
    ps_s = ps[order]
    newgrp = np.empty(E, dtype=bool)
    newgrp[0] = True
    newgrp[1:] = pd_s[1:] != pd_s[:-1]
    gidx = np.nonzero(newgrp)[0]
    rank = np.arange(E) - gidx[np.cumsum(newgrp) - 1]

    lane = pd_s % 128
    core = pd_s // NPC
    slot = (pd_s % NPC) // 128

    nblocks = NP // 128
    KB = np.zeros(nblocks, dtype=np.int64)
    np.maximum.at(KB, pd_s // 128, rank + 1)
    Khat = np.zeros(BPC, dtype=np.int64)
    for k in range(BPC):
        Khat[k] = int(KB[[c * BPC + k for c in range(C)]].max())
    off = np.zeros(BPC + 1, dtype=np.int64)
    off[1:] = np.cumsum(Khat)

    # [core, round-slot, lane] source tables: phase B (tabHS identity rows)
    # and phase C (tab2 zone rows); same rank schedule for both.
    idxB = np.full((C, off[-1], 128), PADIDX, dtype=np.int16)
    idxC = np.full((C, off[-1], 128), PADIDX, dtype=np.int16)
    gslot = off[slot] + rank
    idxB[core, gslot, lane] = (ps_s - FBASE).astype(np.int16)
    idxC[core, gslot, lane] = (_rowmap2(ps_s) - FBASE).astype(np.int16)

    # chunk schedule per block (shared by B and C)
    chunks = []   # (k, r0, ch)
    for k in range(BPC):
        for (r0, ch) in _chunked(int(Khat[k]), CH_MAX):
            chunks.append((k, r0, ch))

    # degp per core [128, BPC]
    degp = np.full(NP, 1e30, dtype=np.float32)
    degp[real_mask] = (deg[final_perm[real_mask]] + 1).astype(np.float32)
    degp_core = degp.reshape(C, BPC, 128).transpose(0, 2, 1).copy()

    # ---- decode: 4 endpoint-zone groups AA, AB, BA, BB ----
    pa = invpos[la]
    pb = invpos[lb]
    LPC = L // C
    za = _zzone(pa)
    zb = _zzone(pb)
    grp_of = za.astype(np.int64) * 2 + zb.astype(np.int64)  # 0=AA 1=AB 2=BA 3=BB
    grp_lists = []   # per core: list of 4 arrays of local label idx
    for c in range(C):
        s = slice(c * LPC, (c + 1) * LPC)
        g = grp_of[s]
        grp_lists.append([np.nonzero(g == i)[0] for i in range(4)])
    rounds_g = []
    chunks_g = []
    for i in range(4):
        r = (max(len(gl[i]) for gl in grp_lists) + 127) // 128
        rounds_g.append(r)
        chunks_g.append(_chunked(r, DEC_CHUNK))

    # per-core slot map: label i (within core slice) -> flat output slot
    dec_slots = []
    for c in range(C):
        sl = np.empty(LPC, dtype=np.int64)
        base = 0
        for i in range(4):
            lst = grp_lists[c][i]
            sl[lst] = base + np.arange(len(lst))
            base += rounds_g[i] * 128
        dec_slots.append(sl)

    def zidx(pos):
        """final positions -> (int16 idx in the proper ztab frame)."""
        zz = _zzone(pos)
        r = _rowmapz(pos)
        return np.where(zz, r, r - FBASE).astype(np.int16)

    padrowA = np.full(128, PADZA, dtype=np.int16)
    padrowB = np.full(128, PADZB, dtype=np.int16)

    def dec_group_tables(c, i, lst):
        s0 = c * LPC
        rounds = rounds_g[i]
        az, bz = i // 2, i % 2
        fa = np.full(rounds * 128, PADZA if az == 0 else PADZB, dtype=np.int16)
        fb = np.full(rounds * 128, PADZA if bz == 0 else PADZB, dtype=np.int16)
        fa[:len(lst)] = zidx(pa[s0 + lst])
        fb[:len(lst)] = zidx(pb[s0 + lst])
        parts = []
        for (r0, ch) in chunks_g[i]:
            parts.append(_wrap_idx(np.concatenate(
                [fa[r0 * 128:(r0 + ch) * 128], padrowA if az == 0 else padrowB])))
            parts.append(_wrap_idx(np.concatenate(
                [fb[r0 * 128:(r0 + ch) * 128], padrowA if bz == 0 else padrowB])))
        return parts

    padrow = np.full(128, PADIDX, dtype=np.int16)
    idx16 = []
    for c in range(C):
        parts = []
        for (k, r0, ch) in chunks:
            flat = idxB[c, off[k] + r0: off[k] + r0 + ch].reshape(-1)
            parts.append(_wrap_idx(np.concatenate([flat, padrow])))
        for (k, r0, ch) in chunks:
            flat = idxC[c, off[k] + r0: off[k] + r0 + ch].reshape(-1)
            parts.append(_wrap_idx(np.concatenate([flat, padrow])))
        for i in range(4):
            parts += dec_group_tables(c, i, grp_lists[c][i])
        idx16.append(np.ascontiguousarray(np.concatenate(parts, axis=1)))

    return dict(
        final_perm=final_perm, invpos=invpos, real_mask=real_mask,
        Khat=Khat, off=off, chunks=chunks,
        chunks_g=chunks_g, rounds_g=rounds_g,
        dec_slots=dec_slots, degp=degp, degp_core=degp_core, idx16=idx16,
    )


def _build(prep):
    chunks = prep["chunks"]
    chunks_g = prep["chunks_g"]
    TOTW = prep["idx16"][0].shape[1]
    ndec_cols = sum(prep["rounds_g"])               # slot s at (s%128, s//128)

    nc = bass.Bass(num_devices=C, dynamic_dma_scratch_size=32768, num_swdge_queues=4)
    f32 = mybir.dt.float32
    xT_d = nc.dram_tensor("xT", [IN, NPC], BF16, kind="ExternalInput")
    W1_d = nc.dram_tensor("W1", [IN, HID], BF16, kind="ExternalInput")
    b1_d = nc.dram_tensor("b1", [1, HID], f32, kind="ExternalInput")
    W2_d = nc.dram_tensor("W2", [HID, OUT], BF16, kind="ExternalInput")
    b2_d = nc.dram_tensor("b2", [1, OUT], f32, kind="ExternalInput")
    degp_d = nc.dram_tensor("degp", [128, BPC], f32, kind="ExternalInput")
    idx_d = nc.dram_tensor("idx16", [128, TOTW], mybir.dt.int16, kind="ExternalInput")
    out_d = nc.dram_tensor("out", [128, ndec_cols], f32, kind="ExternalOutput")

    aghs_in = nc.dram_tensor("aghs_in", [NPC, HID], BF16)
    tabHS = nc.dram_tensor("tabHS", [NP, HID], BF16, addr_space="Shared")
    ag2_in = nc.dram_tensor("ag2_in", [NPC, HID], BF16)
    tab2 = nc.dram_tensor("tab2", [NP, HID], BF16, addr_space="Shared")
    ag3_in = nc.dram_tensor("ag3_in", [NPC, OUT], f32)
    ztabA = nc.dram_tensor("ztabA", [NZA, OUT], f32, addr_space="Shared")
    ztabB = nc.dram_tensor("ztabB", [NZB, OUT], f32, addr_space="Shared")

    grp8 = [list(range(C))]

    with TileContext(nc) as tc, ExitStack() as ctx:
        const = ctx.enter_context(tc.tile_pool(name="const", bufs=1))
        own = ctx.enter_context(tc.tile_pool(name="own", bufs=1))
        xp_pool = ctx.enter_context(tc.tile_pool(name="xpan", bufs=2))
        gp = ctx.enter_context(tc.tile_pool(name="gath", bufs=5))
        dgp = ctx.enter_context(tc.tile_pool(name="dgath", bufs=3))
        ppa = ctx.enter_context(tc.tile_pool(name="psA", bufs=2, space="PSUM"))
        pp = ctx.enter_context(tc.tile_pool(name="psAgg", bufs=3, space="PSUM"))
        pz = ctx.enter_context(tc.tile_pool(name="psZ", bufs=1, space="PSUM"))
        sp_ = ctx.enter_context(tc.tile_pool(name="stage", bufs=4))
        prp = ctx.enter_context(tc.tile_pool(name="prodp", bufs=2))

        ll = nc.gpsimd.load_library(mlp)

        ident = const.tile([128, 128], BF16)
        make_identity(nc, ident[:])
        identf = const.tile([128, 128], mybir.dt.float32)
        make_identity(nc, identf[:])

        idx_sb = const.tile([128, TOTW], mybir.dt.int16)
        idma = nc.sync.dma_start(out=idx_sb[:], in_=idx_d[:, :])
        add_dep_helper(idma.ins, ll.ins, reason="idx after lib load")

        kvals = sorted({(ch + 1) * 128 for (_, _, ch) in chunks}
                       | {(ch + 1) * 128 for cg in chunks_g for (_, ch) in cg})
        kreg = {}
        for v in kvals:
            r = ctx.enter_context(nc.gpsimd.register(f"nidx{v}"))
            nc.gpsimd.reg_mov(r, v)
            kreg[v] = r

        W1_sb = []
        for i in range(2):
            w1t = const.tile([128, HID], BF16, tag=f"w1_{i}", name=f"w1_{i}")
            nc.sync.dma_start(out=w1t[:], in_=W1_d[i * 128:(i + 1) * 128, :])
            W1_sb.append(w1t)
        W2_sb = const.tile([128, OUT], BF16)
        nc.sync.dma_start(out=W2_sb[:], in_=W2_d[:, :])

        ones_row = const.tile([1, 128], f32)
        nc.vector.memset(ones_row[:], 1.0)
        b1_row = const.tile([1, HID], f32)
        nc.sync.dma_start(out=b1_row[:], in_=b1_d[:, :])
        b2_row = const.tile([1, OUT], f32)
        nc.sync.dma_start(out=b2_row[:], in_=b2_d[:, :])
        bias1 = const.tile([128, HID], f32)
        bps = pz.tile([128, HID], f32, tag="qt")
        nc.tensor.matmul(out=bps[:], lhsT=ones_row[:], rhs=b1_row[:], start=True, stop=True)
        nc.scalar.activation(out=bias1[:], in_=bps[:], func=mybir.ActivationFunctionType.Copy)
        bias2 = const.tile([128, OUT], f32)
        bps2 = pz.tile([128, OUT], f32, tag="qt")
        nc.tensor.matmul(out=bps2[:], lhsT=ones_row[:], rhs=b2_row[:], start=True, stop=True)
        nc.scalar.activation(out=bias2[:], in_=bps2[:], func=mybir.ActivationFunctionType.Copy)

        degp_sb = const.tile([128, BPC], f32)
        nc.sync.dma_start(out=degp_sb[:], in_=degp_d[:, :])
        rec = const.tile([128, BPC], f32)
        nc.vector.reciprocal(out=rec[:], in_=degp_sb[:])
        dinv = const.tile([128, BPC], f32)
        nc.scalar.activation(out=dinv[:], in_=rec[:], func=mybir.ActivationFunctionType.Sqrt)

        hs_own = own.tile([128, NPC], BF16)
        g_own = own.tile([128, NPC], BF16)
        z_own = own.tile([128, NPC // 2], f32)   # 49 blocks x 64 cols

        # chunk -> idx column offsets (B group, C group, decode groups)
        co = 0
        blk_chunksB = [[] for _ in range(BPC)]
        for (k, r0, ch) in chunks:
            blk_chunksB[k].append((co, ch))
            co += (ch + 1) * 8
        blk_chunksC = [[] for _ in range(BPC)]
        for (k, r0, ch) in chunks:
            blk_chunksC[k].append((co, ch))
            co += (ch + 1) * 8
        dec_coffs = []   # per group: list of col offsets (a at coff, b adjacent)
        for i in range(4):
            offs = []
            for (r0, ch) in chunks_g[i]:
                offs.append(co)
                co += 2 * (ch + 1) * 8
            dec_coffs.append(offs)
        assert co == TOTW

        # ---------------- Phase A: own-panel GEMM1 -> AllGather hs ----------------
        GW = 3200
        with nc.named_scope("gemm1"):
            for (o0, nb) in ((0, 25), (GW, BPC - 25)):
                x0 = xp_pool.tile([128, GW], BF16, tag="x0", name="x0")
                nc.sync.dma_start(out=x0[:, :nb * 128],
                                  in_=xT_d[0:128, o0: o0 + nb * 128])
                x1 = xp_pool.tile([128, GW], BF16, tag="x1", name="x1")
                nc.sync.dma_start(out=x1[:, :nb * 128],
                                  in_=xT_d[128:256, o0: o0 + nb * 128])
                for q0 in range(0, nb, 4):
                    G = min(4, nb - q0)
                    ps = ppa.tile([128, 512], f32, tag="a", name="psa")
                    for g in range(G):
                        cs = slice((q0 + g) * 128, (q0 + g + 1) * 128)
                        nc.tensor.matmul(out=ps[:, g * 128:(g + 1) * 128],
                                         lhsT=x0[:, cs], rhs=W1_sb[0][:],
                                         start=True, stop=False)
                        nc.tensor.matmul(out=ps[:, g * 128:(g + 1) * 128],
                                         lhsT=x1[:, cs], rhs=W1_sb[1][:],
                                         start=False, stop=True)
                    dst = hs_own[:, o0 + q0 * 128: o0 + (q0 + G) * 128]
                    if (q0 // 4) % 2 == 0:
                        nc.scalar.activation(out=dst, in_=ps[:, :G * 128],
                                             func=mybir.ActivationFunctionType.Copy)
                    else:
                        nc.vector.tensor_copy(out=dst, in_=ps[:, :G * 128])
            whs = nc.sync.dma_start(
                out=aghs_in[:, :].rearrange("(b l) f -> l b f", l=128),
                in_=hs_own[:].rearrange("p (b f) -> p b f", f=HID))
            with nc.named_scope("aghs"):
                cc_hs = nc.gpsimd.collective_compute(
                    "AllGather", mybir.AluOpType.bypass,
                    replica_groups=grp8,
                    ins=[aghs_in[:, :].opt()], outs=[tabHS[:, :].opt()])
                add_dep_helper(cc_hs.ins, whs.ins, reason="aghs")

        qctr = [0]

        def aggregate(k, tab, blk_chunks, own_tile, dep_ins):
            ps = pp.tile([128, HID], f32, tag="main", name="psagg")
            first = True
            for ci, (coff, ch) in enumerate(blk_chunks[k]):
                gt = gp.tile([128, CH_MAX + 1, HID], BF16, tag="gt", name="gt")
                qctr[0] = (qctr[0] + 1) % 4
                gi = nc.gpsimd.dma_gather(
                    gt[:, :ch + 1, :], tab[FBASE:, :],
                    idx_sb[:, coff:coff + (ch + 1) * 8],
                    (ch + 1) * 128, kreg[(ch + 1) * 128], HID, single_packet=False,
                    queue_num=qctr[0])
                for d in dep_ins:
                    add_dep_helper(gi.ins, d.ins, reason="gather after table ready")
                for r in range(ch):
                    nc.tensor.matmul(out=ps[:], lhsT=ident[:], rhs=gt[:, r, :],
                                     start=first, stop=False)
                    first = False
            nc.tensor.matmul(out=ps[:], lhsT=ident[:],
                             rhs=own_tile[:, k * 128:(k + 1) * 128],
                             start=first, stop=True)
            return ps

        # ---------------- Phase B: layer-1 aggregation (tabHS) -> g ----------------
        # Blocks run 48..0 (low-degree first) so ag2 pieces launch early.
        ccs2 = []

        def ag2_stage(o0, o1):
            return nc.sync.dma_start(
                out=ag2_in[o0:o1, :].rearrange("(b l) f -> l b f", l=128),
                in_=g_own[:, o0:o1].rearrange("p (b f) -> p b f", f=HID))

        def ag2_trigger(o0, o1, obase, name, w):
            with nc.named_scope(name):
                cc = nc.gpsimd.collective_compute(
                    "AllGather", mybir.AluOpType.bypass,
                    replica_groups=grp8,
                    ins=[ag2_in[o0:o1, :].opt()],
                    outs=[tab2[obase:obase + C * (o1 - o0), :].opt()])
                add_dep_helper(cc.ins, w.ins, reason=name)
                ccs2.append(cc)

        with nc.named_scope("agg1"):
            w2p1 = None
            for i, k in enumerate(reversed(range(BPC))):
                dep = [cc_hs] if i == 0 else []
                ps = aggregate(k, tabHS, blk_chunksB, hs_own, dep)
                t1 = sp_.tile([128, HID], f32, tag="t1", name="t1")
                nc.scalar.activation(out=t1[:], in_=ps[:],
                                     func=mybir.ActivationFunctionType.Copy,
                                     scale=dinv[:, k:k + 1])
                t2 = sp_.tile([128, HID], f32, tag="t2", name="t2")
                nc.vector.tensor_add(out=t2[:], in0=t1[:], in1=bias1[:])
                nc.scalar.activation(out=g_own[:, k * 128:(k + 1) * 128], in_=t2[:],
                                     func=mybir.ActivationFunctionType.Relu,
                                     scale=dinv[:, k:k + 1])
                if k == P1_O // 128 + STAGE_AHEAD:    # stage piece 1 early
                    w2p1 = ag2_stage(P1_O + STAGE_AHEAD * 128, NPC)
                if k == P1_O // 128:                  # blocks 48..28 done
                    w2p1b = ag2_stage(P1_O, P1_O + STAGE_AHEAD * 128)
                    add_dep_helper(w2p1b.ins, w2p1.ins, reason="order stage")
                    ag2_trigger(P1_O, NPC, R1_BASE, "ag2p1", w2p1b)
                if k == P2_O // 128 + STAGE_AHEAD:    # stage piece 2 early
                    w2p2 = ag2_stage(P2_O + STAGE_AHEAD * 128, P1_O)
                if k == P2_O // 128:                  # blocks 27..4 done
                    w2p2b = ag2_stage(P2_O, P2_O + STAGE_AHEAD * 128)
                    add_dep_helper(w2p2b.ins, w2p2.ins, reason="order stage")
                    ag2_trigger(P2_O, P1_O, R2_BASE, "ag2p2", w2p2b)
            w2p3 = ag2_stage(0, P2_O)
            ag2_trigger(0, P2_O, R3_BASE, "ag2p3", w2p3)

        # ---------------- Phase C: layer-2 aggregation + GEMM2 -> z ----------------
        ccs3 = []
        with nc.named_scope("agg2"):
            for i, k in enumerate(reversed(range(BPC))):
                dep = ccs2 if i == 0 else []
                ps = aggregate(k, tab2, blk_chunksC, g_own, dep)
                q = sp_.tile([128, HID], f32, tag="q", name="q")
                nc.scalar.activation(out=q[:], in_=ps[:],
                                     func=mybir.ActivationFunctionType.Copy,
                                     scale=dinv[:, k:k + 1])
                qt_ps = pz.tile([128, HID], f32, tag="qt", name="qtps")
                nc.tensor.transpose(out=qt_ps[:], in_=q[:], identity=identf[:])
                qt = sp_.tile([128, HID], BF16, tag="qt_sb", name="qtsb")
                nc.vector.tensor_copy(out=qt[:], in_=qt_ps[:])
                zps = pz.tile([128, OUT], f32, tag="z", name="zps")
                nc.tensor.matmul(out=zps[:], lhsT=qt[:], rhs=W2_sb[:], start=True, stop=True)
                nc.vector.tensor_add(out=z_own[:, k * OUT:(k + 1) * OUT],
                                     in0=zps[:], in1=bias2[:])
                if k == ZB_O // 128 + STAGE_AHEAD:    # stage most of ztabA early
                    w3a = nc.sync.dma_start(
                        out=ag3_in[ZB_O + STAGE_AHEAD * 128:NPC, :].rearrange(
                            "(b l) f -> l b f", l=128),
                        in_=z_own[:, (ZB_O // 128 + STAGE_AHEAD) * OUT:].rearrange(
                            "p (b f) -> p b f", f=OUT))
                if k == ZB_O // 128:          # blocks 48..14 done -> ztabA piece
                    w3ab = nc.sync.dma_start(
                        out=ag3_in[ZB_O:ZB_O + STAGE_AHEAD * 128, :].rearrange(
                            "(b l) f -> l b f", l=128),
                        in_=z_own[:, (ZB_O // 128) * OUT:
                                  (ZB_O // 128 + STAGE_AHEAD) * OUT].rearrange(
                            "p (b f) -> p b f", f=OUT))
                    add_dep_helper(w3ab.ins, w3a.ins, reason="order stage")
                    with nc.named_scope("ag3a"):
                        cc = nc.gpsimd.collective_compute(
                            "AllGather", mybir.AluOpType.bypass,
                            replica_groups=grp8,
                            ins=[ag3_in[ZB_O:NPC, :].opt()],
                            outs=[ztabA[:, :].opt()])
                        add_dep_helper(cc.ins, w3ab.ins, reason="ag3a")
                        ccs3.append(cc)
            w3b = nc.sync.dma_start(
                out=ag3_in[0:ZB_O, :].rearrange("(b l) f -> l b f", l=128),
                in_=z_own[:, :(ZB_O // 128) * OUT].rearrange("p (b f) -> p b f", f=OUT))

        # ---------------- Phase D: decode (4 endpoint-zone groups) ----------------
        # Emit the first AA chunk's gathers BEFORE the ag3b trigger so its
        # staging wait overlaps decode gather time.
        with nc.named_scope("decode"):
            out_sb = own.tile([128, ndec_cols], f32)

            def dec_chunk(i, ci, col, deps_a=(), deps_b=()):
                (r0, ch) = chunks_g[i][ci]
                coff = dec_coffs[i][ci]
                az, bz = i // 2, i % 2
                taba = ztabA[FBASE:, :] if az == 0 else ztabB[0:, :]
                tabb = ztabA[FBASE:, :] if bz == 0 else ztabB[0:, :]
                qa = (2 * ci) % 4
                qb = (2 * ci + 1) % 4
                za_t = dgp.tile([128, DEC_CHUNK + 1, OUT], f32, tag="za", name="za")
                ga = nc.gpsimd.dma_gather(
                    za_t[:, :ch + 1, :], taba,
                    idx_sb[:, coff:coff + (ch + 1) * 8],
                    (ch + 1) * 128, kreg[(ch + 1) * 128], OUT, single_packet=False,
                    queue_num=qa)
                for dcc in deps_a:
                    add_dep_helper(ga.ins, dcc.ins, reason="dec a after ztab")
                zb_t = dgp.tile([128, DEC_CHUNK + 1, OUT], f32, tag="zb", name="zb")
                gb = nc.gpsimd.dma_gather(
                    zb_t[:, :ch + 1, :], tabb,
                    idx_sb[:, coff + (ch + 1) * 8:coff + 2 * (ch + 1) * 8],
                    (ch + 1) * 128, kreg[(ch + 1) * 128], OUT, single_packet=False,
                    queue_num=qb)
                for dcc in deps_b:
                    add_dep_helper(gb.ins, dcc.ins, reason="dec b after ztab")
                prod = prp.tile([128, ch * OUT], f32, tag="prod", name="prod")
                nc.vector.tensor_mul(out=prod[:].rearrange("p (c o) -> p c o", o=OUT),
                                     in0=za_t[:, :ch, :], in1=zb_t[:, :ch, :])
                nc.vector.reduce_sum(out=out_sb[:, col:col + ch],
                                     in_=prod[:].rearrange("p (c o) -> p c o", o=OUT),
                                     axis=mybir.AxisListType.X)
                return col + ch

            with nc.named_scope("ag3b"):
                cc = nc.gpsimd.collective_compute(
                    "AllGather", mybir.AluOpType.bypass,
                    replica_groups=grp8,
                    ins=[ag3_in[0:ZB_O, :].opt()], outs=[ztabB[:, :].opt()])
                add_dep_helper(cc.ins, w3b.ins, reason="ag3b")
                ccs3.append(cc)

            col = 0
            # first AA chunk: explicit deps on the ztabA piece (not ztabB)
            col = dec_chunk(0, 0, col, deps_a=(ccs3[0],), deps_b=(ccs3[0],))
            for ci in range(1, len(chunks_g[0])):
                col = dec_chunk(0, ci, col)
            first_b = True
            for i in (1, 2, 3):
                for ci in range(len(chunks_g[i])):
                    db = (ccs3[1],) if first_b else ()
                    col = dec_chunk(i, ci, col, deps_b=db)
                    first_b = False
            nc.sync.dma_start(out=out_d[:, :], in_=out_sb[:])

    lower_extended_insts(nc)
    _fix_sync_waits(nc)
    return nc


def kernel(x, W1, b1, W2, b2, edge_index, edge_label_index):
    x = np.asarray(x, dtype=np.float32)
    W1 = np.asarray(W1, dtype=np.float32)
    b1 = np.asarray(b1, dtype=np.float32)
    W2 = np.asarray(W2, dtype=np.float32)
    b2 = np.asarray(b2, dtype=np.float32)
    prep = _prepare(np.asarray(edge_index), np.asarray(edge_label_index))
    nc = _build(prep)

    # host-prescaled x: xp[pos] = dinv[pos] * x[perm[pos]]  (pads stay zero)
    xp = np.zeros((NP, IN), dtype=np.float32)
    rm = prep["real_mask"]
    xp[rm] = x[prep["final_perm"][rm]]
    dinv_full = 1.0 / np.sqrt(prep["degp"])
    xp *= dinv_full[:, None]

    import ml_dtypes

    def to_bf16(a):
        return np.asarray(a, dtype=np.float32).astype(ml_dtypes.bfloat16)

    in_maps = []
    for c in range(C):
        xc = xp[c * NPC:(c + 1) * NPC]
        in_maps.append({
            "xT": to_bf16(np.ascontiguousarray(xc.T)),
            "W1": to_bf16(W1), "b1": b1.reshape(1, HID),
            "W2": to_bf16(W2), "b2": b2.reshape(1, OUT),
            "degp": prep["degp_core"][c],
            "idx16": prep["idx16"][c],
        })
    res = run_bass_kernel_spmd(nc, in_maps, core_ids=list(range(C)))

    LPC = L // C
    out = np.empty(L, dtype=np.float32)
    for c in range(C):
        o = res.results[c]["out"]          # [128, ncols]; slot s at (s%128, s//128)
        sl = prep["dec_slots"][c]
        out[c * LPC:(c + 1) * LPC] = o[sl % 128, sl // 128]
    return out
